# revision 30
# baseline (speedup 1.0000x reference)
"""DisparityFusion Trainium2 kernel (8 NeuronCores, SPMD data-parallel), v2.

Full inputs in, full output out. Sharding: core c handles batch b=c//4 and
output rows [64*(c%4), 64*(c%4)+64), with a 1-row halo computed locally.

v2 strategy vs baseline (216us):
  * Cost volumes shipped as fp8 e4m3 (halves HBM traffic).
  * exp() split across TWO engines: ScalarE does true exp with fp8 output
    (rows 0..15, 48..65); VectorE does rows 16..47 via the int-bitcast
    fast-exp trick: v = round(x*8/ln2 + 39.65) written as int16 whose LOW
    byte is a valid e4m3 bit pattern for ~e^(x-ln4) (sawtooth rel err ~±5%,
    cancels in the softmax ratio to first order). The fp8 values are read
    with a stride-2 byte view. Host clamps x to [-3.25, 5.5] on the DVE rows
    so v stays in [0, 119] (no NaN/Inf bit patterns).
  * All stage-1 reductions are fp8 DoubleRow matmuls (2 elem/cycle): per
    32-row block one PSUM tile [96,512] holds (s0, s1a, s1b) interleaved at
    partitions 3r+{0,1,2}; stationary weights (1, 16*(d>>4), d&15) are all
    exactly representable in e4m3; s1 = s1a + s1b on VectorE.
  * No boundary mask: out-of-image halo rows get a host "spike" pattern
    (x[0]=+5.5, x[1:]=-15) whose regression is exactly 0 after fp8 exp.
  * Stage 2: per-chunk [4,512] sum matmuls widened to one [64,512] psa/pss
    accumulation (16 shifted-column stationaries), single den/recip/mul and
    ONE output DMA. Patch product on GpSimd, affinity ReLU on ScalarE.
"""

import math
import sys

sys.path.insert(0, "/opt/trn_rl_repo")

from contextlib import ExitStack

import numpy as np
import ml_dtypes

import concourse.bass as bass
import concourse.bacc as bacc
import concourse.tile as tile
from concourse import mybir
from concourse import bass_utils

B, D, H, W = 2, 192, 256, 512
N_CORES = 8
SLAB = 64            # output rows per core
SLABP = SLAB + 2     # slab + 1-row halo each side
BLOCKS = [(0, 32), (32, 32), (64, 2)]  # (r0, nr) over the 66 slab rows

# exp-path row split (A rows; B packed rows j cover images rows 2j,2j+1)
DVE_R0, DVE_R1 = 16, 48          # rows 16..47 -> VectorE fast-exp
LN4 = math.log(4.0)
K_TRICK = 8.0 / math.log(2.0)            # 11.5416
C_TRICK = 8.0 * (7.0 - 2.0 - 0.04367)    # 39.6506
X_LO, X_HI = -3.25, 5.5

F32 = mybir.dt.float32
F16 = mybir.dt.float16
F32R = mybir.dt.float32r
F8 = mybir.dt.float8e4
I16 = mybir.dt.int16
FP8_NP = ml_dtypes.float8_e4m3
DR = mybir.MatmulPerfMode.DoubleRow
AOP = mybir.AluOpType


def _build_nc():
    nc = bacc.Bacc(
        "TRN2",
        target_bir_lowering=False,
        debug=False,
        enable_asserts=False,
        num_devices=N_CORES,
    )

    xas = [
        nc.dram_tensor(f"xa{i}", [128, SLABP, W], F8, kind="ExternalInput").ap()
        for i in (1, 2, 3)
    ]
    xbs = [
        nc.dram_tensor(f"xb{i}", [128, SLABP // 2, W], F8, kind="ExternalInput").ap()
        for i in (1, 2, 3)
    ]
    laP_d = nc.dram_tensor("laP", [128, 16, 2, 96], F8, kind="ExternalInput").ap()
    lbP_d = nc.dram_tensor("lbP", [128, 8, 2, 96], F8, kind="ExternalInput").ap()
    lbs_d = nc.dram_tensor("lbs", [128, 96], F8, kind="ExternalInput").ap()
    wc_d = nc.dram_tensor("wc", [108, 108], F32, kind="ExternalInput").ap()
    wsA_d = nc.dram_tensor("wsA", [108, 16, 64], F32, kind="ExternalInput").ap()
    whA_d = nc.dram_tensor("whA", [108, 16, 64], F32, kind="ExternalInput").ap()
    bv_d = nc.dram_tensor("bv", [108, 1], F32, kind="ExternalInput").ap()
    z1_d = nc.dram_tensor("z1", [1, 1], F32, kind="ExternalInput").ap()
    out_d = nc.dram_tensor("out", [SLAB, W], F32, kind="ExternalOutput").ap()

    with tile.TileContext(nc) as tc, ExitStack() as ctx:
        consts = ctx.enter_context(tc.tile_pool(name="consts", bufs=1))
        dpads = ctx.enter_context(tc.tile_pool(name="dpads", bufs=1))

        laP = consts.tile([128, 16, 2, 96], F8, tag="laP")
        nc.gpsimd.dma_start(laP[:], laP_d[:])
        lbP = consts.tile([128, 8, 2, 96], F8, tag="lbP")
        nc.gpsimd.dma_start(lbP[:], lbP_d[:])
        lbs = consts.tile([128, 96], F8, tag="lbs")
        nc.gpsimd.dma_start(lbs[:], lbs_d[:])
        wc = consts.tile([108, 108], F16, tag="wc")
        nc.gpsimd.dma_start(wc[:], wc_d[:])
        wsA = consts.tile([108, 16, 64], F32R, tag="wsA")
        nc.gpsimd.dma_start(wsA[:], wsA_d[:])
        whA = consts.tile([108, 16, 64], F16, tag="whA")
        nc.gpsimd.dma_start(whA[:], whA_d[:])
        bv = consts.tile([108, 1], F32, tag="bv")
        nc.gpsimd.dma_start(bv[:], bv_d[:])
        eps64 = consts.tile([64, 1], F32, tag="eps64")
        nc.vector.memset(eps64[:], 1e-8)
        nln4 = consts.tile([128, 1], F32, tag="nln4")
        nc.vector.memset(nln4[:], -LN4)
        warm3 = consts.tile([32, 3], F32, tag="warm3")
        nc.vector.memset(warm3[:], 0.25)
        warm = consts.tile([32, 3], F32, tag="warm")
        nc.scalar.activation(warm[:], warm3[:], mybir.ActivationFunctionType.Exp)

        dps = []
        for i in range(3):
            dp = dpads.tile([SLABP, W + 2], F16, tag=f"dp{i}")
            nc.vector.memset(dp[:], 0.0)
            dps.append(dp)

        im_p = ctx.enter_context(tc.tile_pool(name="im", bufs=1))
        # IM partition layout: p = (9*br + tap)*4 + q  (q = 16-row quarter)
        im_r = im_p.tile([108, 16, W], F16, tag="imr")
        # 4-byte seed write: forces slot allocation before the sliced im2col
        # writes that the scheduler otherwise mis-tracks across pool scopes
        nc.gpsimd.dma_start(im_r[0:1, 0:1, 0:1], z1_d[:, :])

        # ---------------- Stage 1: softmax-expectation ----------------
        with ExitStack() as s1:
            xs_p = s1.enter_context(tc.tile_pool(name="xs", bufs=3))
            es_p = s1.enter_context(tc.tile_pool(name="es", bufs=4))
            xv_p = s1.enter_context(tc.tile_pool(name="xv", bufs=3))
            ev_p = s1.enter_context(tc.tile_pool(name="ev", bufs=3))
            xsb_p = s1.enter_context(tc.tile_pool(name="xsb", bufs=2))
            esb_p = s1.enter_context(tc.tile_pool(name="esb", bufs=4))
            xvb_p = s1.enter_context(tc.tile_pool(name="xvb", bufs=2))
            evb_p = s1.enter_context(tc.tile_pool(name="evb", bufs=4))
            ps_p = s1.enter_context(tc.tile_pool(name="ps1", bufs=3, space="PSUM"))
            dv_p = s1.enter_context(tc.tile_pool(name="div", bufs=2))

            for br in range(3):
                xa = xas[br]
                xb = xbs[br]
                dp = dps[br]

                # --- produce e-tiles for this branch ---
                # A-chunk DMA segments: (start_row, n_rows, is_dve)
                a_segs = [(0, 16, 0), (16, 16, 1), (32, 16, 1), (48, 16, 0),
                          (64, 2, 0)]
                # B-chunk DMA segments in packed rows (start_j, n_j, is_dve)
                b_segs = [(0, 8, 0), (8, 8, 1), (16, 8, 1), (24, 8, 0),
                          (32, 1, 0)]

                ea = {}   # start_row -> ap [128, n, 512] fp8
                eb = {}   # start_j  -> ap [128, n, 512] fp8

                def _exp_tiles(segs, src_ap, out, sub, dve_pools, sc_pools):
                    xv_pool, ev_pool = dve_pools
                    xs_pool, es_pool = sc_pools
                    for g, (r0, n, is_dve) in enumerate(segs):
                        xt_pool = xv_pool if is_dve else xs_pool
                        xt = xt_pool.tile([128, n, W], F8, tag=f"x{n}{is_dve}")
                        q = nc.gpsimd if is_dve else nc.sync
                        q.dma_start(xt[:], src_ap[:, r0 : r0 + n, :])
                        for c0 in range(0, n, sub):
                            c1 = min(c0 + sub, n)
                            m = c1 - c0
                            if is_dve:
                                ei = ev_pool.tile([128, m, W], I16, tag=f"e{m}")
                                nc.vector.tensor_scalar(
                                    ei[:], xt[:, c0:c1],
                                    K_TRICK, C_TRICK, AOP.mult, AOP.add,
                                )
                                ev = ei[:].bitcast(F8).rearrange(
                                    "p n (w c) -> p n w c", c=2
                                )
                                out[r0 + c0] = ev[:, :, :, 0]
                            else:
                                es = es_pool.tile([128, m, W], F8, tag=f"s{m}")
                                nc.scalar.activation(
                                    es[:], xt[:, c0:c1],
                                    mybir.ActivationFunctionType.Exp,
                                    bias=nln4[:],
                                )
                                out[r0 + c0] = es[:]

                _exp_tiles(a_segs, xa, ea, 8, (xv_p, ev_p), (xs_p, es_p))
                _exp_tiles(b_segs, xb, eb, 4, (xvb_p, evb_p), (xsb_p, esb_p))

                def _lookup(table, idx):
                    for s0 in sorted(table, reverse=True):
                        if s0 <= idx:
                            return table[s0][:, idx - s0 : idx - s0 + 2, :]
                    raise AssertionError(idx)

                a_pair = lambda r: _lookup(ea, r)
                b_pair = lambda j: _lookup(eb, j)

                # --- blocks: accumulate (s0, s1a, s1b) at partitions 3r+t ---
                for blk, (r0, nr) in enumerate(BLOCKS):
                    ps = ps_p.tile([96, W], F32, tag="ps1")
                    if nr == 32:
                        n_mm = 24
                        k = 0
                        # chunk A pairs
                        for r in range(r0, r0 + nr, 2):
                            rl = (r - r0) // 2
                            nc.tensor.matmul(
                                ps[:], laP[:, rl], a_pair(r),
                                start=(k == 0), stop=False, perf_mode=DR,
                            )
                            k += 1
                        # chunk B packed pairs (j, j+1) -> rows 2j..2j+3
                        for j in range(r0 // 2, (r0 + nr) // 2, 2):
                            jl = (j - r0 // 2) // 2
                            k += 1
                            nc.tensor.matmul(
                                ps[:], lbP[:, jl], b_pair(j),
                                start=False, stop=(k == n_mm), perf_mode=DR,
                            )
                    else:
                        # tail block: rows 64,65 = A pair + single packed row
                        nc.tensor.matmul(
                            ps[:], laP[:, 0], a_pair(64),
                            start=True, stop=False, perf_mode=DR,
                        )
                        nc.tensor.matmul(
                            ps[:], lbs[:], eb[32][:, 0, :],
                            start=False, stop=True,
                        )

                    rec = dv_p.tile([32, W], F32, tag="rec")
                    nc.vector.reciprocal_approx_fast(rec[0:nr], ps[0:nr])
                    t1 = dv_p.tile([32, W], F16, tag="t1")
                    nc.vector.tensor_mul(t1[0:nr], ps[32 : 32 + nr], rec[0:nr])
                    t2 = dv_p.tile([32, W], F16, tag="t2")
                    nc.vector.tensor_mul(t2[0:nr], ps[64 : 64 + nr], rec[0:nr])
                    dt = dv_p.tile([32, W], F16, tag="dt")
                    nc.vector.tensor_add(dt[0:nr], t1[0:nr], t2[0:nr])
                    nc.gpsimd.dma_start(dp[r0 : r0 + nr, 1 : W + 1], dt[0:nr])
                    if br == 2 and blk == 1:
                        for tap in range(9):
                            dy, dx = tap // 3, tap % 3
                            p = (9 * br + tap) * 4
                            eng = nc.sync if tap % 2 == 0 else nc.gpsimd
                            eng.dma_start(
                                im_r[p : p + 2, :, :],
                                dp[dy : dy + 32, dx : dx + W],
                            )

                # patch materialization for this branch as soon as its d_pad
                # completes; br2's lower half was already issued after block 1
                if br < 2:
                    for tap in range(9):
                        dy, dx = tap // 3, tap % 3
                        p = (9 * br + tap) * 4
                        nc.gpsimd.dma_start(
                            im_r[p : p + 4, :, :],
                            dp[dy : dy + 64, dx : dx + W],
                        )
                else:
                    for tap in range(9):
                        dy, dx = tap // 3, tap % 3
                        p = (9 * br + tap) * 4
                        eng = nc.sync if tap % 2 == 0 else nc.gpsimd
                        eng.dma_start(
                            im_r[p + 2 : p + 4, :, :],
                            dp[dy + 32 : dy + 64, dx : dx + W],
                        )

        # ---------------- Stage 2: affinity gates + propagation --------------
        with ExitStack() as s2:
            aff_p = s2.enter_context(tc.tile_pool(name="aff", bufs=4))
            prod_p = s2.enter_context(tc.tile_pool(name="prod", bufs=4))
            pc_p = s2.enter_context(tc.tile_pool(name="pc", bufs=3, space="PSUM"))
            pss_p = s2.enter_context(tc.tile_pool(name="pss", bufs=1, space="PSUM"))
            psa_p = s2.enter_context(tc.tile_pool(name="psa", bufs=1, space="PSUM"))
            fin_p = s2.enter_context(tc.tile_pool(name="fin", bufs=1))

            pss0 = pss_p.tile([32, W], F32, tag="pss0")
            pss1 = pss_p.tile([32, W], F32, tag="pss1")
            psa0 = psa_p.tile([32, W], F32, tag="psa0")
            psa1 = psa_p.tile([32, W], F32, tag="psa1")
            psss = [pss0, pss1]
            psas = [psa0, psa1]
            out_v = out_d.rearrange("(q n) w -> n q w", q=4)

            def _finale(half):
                den = fin_p.tile([32, W], F32, tag=f"den{half}")
                nc.scalar.activation(
                    den[:], psss[half][:],
                    mybir.ActivationFunctionType.Identity, bias=eps64[0:32],
                )
                rec2 = fin_p.tile([32, W], F32, tag=f"rec2{half}")
                nc.vector.reciprocal_approx_fast(rec2[:], den[:])
                oc = fin_p.tile([32, W], F32, tag=f"oc{half}")
                nc.vector.tensor_mul(oc[:], psas[half][:], rec2[:])
                nc.sync.dma_start(out_v[8 * half : 8 * half + 8], oc[:])

            for n in range(16):
                h, c0 = n // 8, 32 * (n // 8)
                pc = pc_p.tile([108, W], F32, tag="pc")
                nc.tensor.matmul(pc[:], wc[:], im_r[:, n, :], start=True, stop=True)
                aff = aff_p.tile([108, W], F16, tag="aff")
                nc.scalar.activation(
                    aff[:], pc[:], mybir.ActivationFunctionType.Relu, bias=bv[:]
                )
                nc.tensor.matmul(
                    psss[h][:], whA[:, n, c0 : c0 + 32], aff[:],
                    start=(n % 8 == 0), stop=(n % 8 == 7),
                )
                prod = prod_p.tile([108, W], F32R, tag="prod")
                peng = nc.vector if n % 2 else nc.gpsimd
                peng.tensor_mul(prod[:], aff[:], im_r[:, n, :])
                nc.tensor.matmul(
                    psas[h][:], wsA[:, n, c0 : c0 + 32], prod[:],
                    start=(n % 8 == 0), stop=(n % 8 == 7),
                )
                if n == 7:
                    _finale(0)
            _finale(1)

    nc.compile()
    return nc


_NC_CACHE = None


def _get_nc():
    global _NC_CACHE
    if _NC_CACHE is None:
        _NC_CACHE = _build_nc()
    return _NC_CACHE


def _host_consts(W1, g1, b1, W2, g2, b2, W3, g3, b3):
    # Stage-1 DoubleRow stationaries. Column order for an A pair (r, r+1):
    # (s0_r, s1a_r, s1b_r, s0_r1, s1a_r1, s1b_r1) -> psum partitions 3r+t.
    # slice i holds row r+i's weights; d(p) = p for chunk A.
    dh16_a = (16 * (np.arange(128) >> 4)).astype(np.float32)
    dl_a = (np.arange(128) & 15).astype(np.float32)
    laP = np.zeros((128, 16, 2, 96), np.float32)
    for rl in range(16):
        for i in range(2):
            laP[:, rl, i, 2 * rl + i] = 1.0
            laP[:, rl, i, 32 + 2 * rl + i] = dh16_a
            laP[:, rl, i, 64 + 2 * rl + i] = dl_a
    # B packed: partition p -> d = 128 + (p % 64); p<64 row 2j, p>=64 row 2j+1
    db = 128 + (np.arange(128) % 64)
    dh16_b = (16 * (db >> 4)).astype(np.float32)
    dl_b = (db & 15).astype(np.float32)
    lo = np.arange(128) < 64
    hi = ~lo
    # DR pair (j, j+1): slice i covers packed row j+i = image rows 2(j+i),+1
    # col order: (s0_2j, s1a_2j, s1b_2j, s0_2j+1, ..., s1b_2j+3)
    lbP = np.zeros((128, 8, 2, 96), np.float32)
    for jl in range(8):
        for i in range(2):
            for half, m in ((0, lo), (1, hi)):
                c = 4 * jl + 2 * i + half
                lbP[m, jl, i, c] = 1.0
                lbP[m, jl, i, 32 + c] = dh16_b[m]
                lbP[m, jl, i, 64 + c] = dl_b[m]
    # single packed row (rows 64, 65), non-DR
    lbs = np.zeros((128, 96), np.float32)
    for half, m in ((0, lo), (1, hi)):
        lbs[m, half] = 1.0
        lbs[m, 32 + half] = dh16_b[m]
        lbs[m, 64 + half] = dl_b[m]

    # Stage-2: k/m space p = (9*br + c)*4 + q
    Ws = [W1, W2, W3]
    gs = [g1, g2, g3]
    bs = [b1, b2, b3]
    wc = np.zeros((108, 108), np.float32)
    wsA = np.zeros((108, 16, 64), np.float32)
    bv = np.zeros((108, 1), np.float32)
    for br in range(3):
        wflat = Ws[br].reshape(9, 9)  # [c, tap]
        for c in range(9):
            for tap in range(9):
                for q in range(4):
                    wc[(9 * br + tap) * 4 + q, (9 * br + c) * 4 + q] = (
                        wflat[c, tap] * gs[br][c]
                    )
        for c in range(9):
            for q in range(4):
                for n in range(16):
                    wsA[(9 * br + c) * 4 + q, n, 4 * n + q] = 1.0
                bv[(9 * br + c) * 4 + q, 0] = bs[br][c]
    f8 = lambda a: a.astype(FP8_NP)
    return f8(laP), f8(lbP), f8(lbs), wc, wsA, bv


def prepare_in_maps(out_1, out_2, out_3, W1, g1, b1, W2, g2, b2, W3, g3, b3):
    xs_full = [np.asarray(a, np.float32) for a in (out_1, out_2, out_3)]
    laP, lbP, lbs, wc, wsA, bv = _host_consts(
        *[np.asarray(a, np.float32) for a in (W1, g1, b1, W2, g2, b2, W3, g3, b3)]
    )

    spike = np.full((D, 1, 1), -15.0, np.float32)
    spike[0] = 5.5

    in_maps = []
    for c in range(N_CORES):
        b = c // 4
        h0 = SLAB * (c % 4)
        lo, hi = max(0, h0 - 1), min(H, h0 + SLAB + 1)

        im = {"laP": laP, "lbP": lbP, "lbs": lbs, "wc": wc,
              "wsA": wsA, "whA": wsA, "bv": bv,
              "z1": np.zeros((1, 1), np.float32)}
        for i, xf in enumerate(xs_full):
            shard = np.empty((D, SLABP, W), np.float32)
            shard[:, lo - (h0 - 1) : hi - (h0 - 1), :] = xf[b, :, lo:hi, :]
            if h0 == 0:
                shard[:, 0:1, :] = spike
            if h0 + SLAB == H:
                shard[:, SLABP - 1 :, :] = spike
            np.clip(shard[:, DVE_R0:DVE_R1, :], X_LO, X_HI,
                    out=shard[:, DVE_R0:DVE_R1, :])
            np.minimum(shard, X_HI, out=shard)
            im[f"xa{i + 1}"] = shard[0:128].astype(FP8_NP)
            cb = shard[128:192].reshape(64, SLABP // 2, 2, W)
            im[f"xb{i + 1}"] = np.ascontiguousarray(
                np.concatenate([cb[:, :, 0, :], cb[:, :, 1, :]], axis=0)
            ).astype(FP8_NP)
        in_maps.append(im)
    return in_maps


def gather(results):
    out = np.zeros((B, H, W), np.float32)
    for c in range(N_CORES):
        b = c // 4
        h0 = SLAB * (c % 4)
        out[b, h0 : h0 + SLAB, :] = results[c]["out"]
    return out


def kernel(**inputs):
    in_maps = prepare_in_maps(**inputs)
    res = bass_utils.run_bass_kernel_spmd(
        _get_nc(), in_maps, core_ids=list(range(N_CORES))
    )
    return gather(res.results)
